# revision 59
# baseline (speedup 1.0000x reference)
"""Multi-head causal attention on 8 Trainium2 NeuronCores.

nn_MultiHeadAttention_37933151158277: x[2,2048,2048] f32, causal mask,
W_qkv[6144,2048], W_o[2048,2048]. Tensor-parallel over heads (2 per
core), per the sharding hint: qkv_proj output and W_o input are split
along the head dimension; x is replicated. Each core:

  phase 1 - QKV projection, all inputs in bf16 (same 1-cycle/row PE
      rate as f32r, half the DMA bytes; quantization adds ~2e-3 to the
      end-to-end error, far under the 2e-2 gate). Host supplies x^T
      pre-tiled ([kd, c] tiles contiguous in DRAM) and per-core weight
      slices pre-transposed, so Q^T/K^T land as [d_k=128, tok] and V
      as [tok, d_k] with zero on-device transposes. Weight DMAs are
      emitted inside the first chunk's kd loop, interleaved with the
      x^T tile loads, so the PE starts after ~one tile of DMA instead
      of the full preload; W_o loads are deferred to phase 2.
  phase 2 - attention per (batch, head). Scores are computed
      transposed: S^T[k, q] = K^T_tile.T @ Q^T (contraction over d), so
      the P @ V matmul can consume exp(S^T) directly with V tiles as
      the stationary operand. No max-subtraction (scores are O(1) by
      construction, exp cannot overflow). The softmax denominator is
      accumulated mostly ON THE PE: an all-ones [128,128] stationary
      matmul per entry into a dedicated PSUM bank yields the
      partition-reduced denominator already broadcast across all 128
      partitions; early full-width entries are offloaded to a GpSimd
      add-chain whose per-partition sum the q-block tail folds into
      the same PSUM accumulation group with one extra ones-matmul.
      The tail is then just reciprocal_approx_fast (~18-bit, on all
      128 DVE lanes) + one o_ps x 1/d multiply - nothing serial
      enough to stall the in-order PE queue, which matters doubly
      because the PE drops out of its 2.4GHz p-state after any gap
      (mid p-state is 1.2GHz for the next 3us). Two head-streams are
      emitted entry-interleaved; 2 proj tiles are held in reserve to
      feed the PE across each block tail.
  phase 3 - partial out-projection y_c = attn_out @ W_o[:, cols]^T,
      drained one q-block behind attention; PSUM->SBUF copies rotate
      between Scalar and DVE so neither queue delays exp/reciprocal;
      y is written in bf16 (halves output DMA; negligible error since
      the host reduces the 8 partial sums in f32).

PSUM budget (8 x 2KB banks): s_ps 2 + o_ps 2 + d_ps 2 + y_ps 2.

Host: y = sum_c y_c (the unshard of the head-parallel partial sums).

The mask is analyzed block-wise at trace time: fully-masked blocks are
skipped, fully-valid blocks skip the mask multiply, mixed blocks get a
(content-deduped) DMA'd mask-tile multiply - for the causal mask this
yields the optimal lower-triangular schedule with a single shared
128x128 triangle tile.

fp8 was measured (numpy e4m3 emulation) and rejected: every variant
(scores-only, PV-only, projections) lands at 0.9e-2..6.4e-2 absmax -
at or over the 2e-2 gate - so matmuls stay bf16/f32r at 1 cycle/row.

Measured: 371999 ns HW exec (8 cores SPMD), rel err 3.0e-3
(baseline inherited from the previous session: 462552 ns / 2.5e-4).
"""
import sys
if '/opt/trn_rl_repo' not in sys.path:
    sys.path.insert(0, '/opt/trn_rl_repo')

import numpy as np

B, S, D = 2, 2048, 2048
H, DK = 16, 128
NCORES = 8
HPC = H // NCORES            # heads per core
T = B * S                    # tokens
QB = 512                     # q-block width (free dim of S^T / PV matmuls)
NKT = S // 128               # k tiles per batch (16)
NQB = S // QB                # q blocks per batch (4)
NCH = T // QB                # token chunks (8)
NDT = D // 128               # d_model tiles (16)
SGRP = 1                     # k-tiles per s-psum group

_cache = {}
DSPLIT = True


def _analyze_mask(m2):
    """m2: [S, S] bool, m2[q, k]. Returns blocks[qb] = list of entries
    (j, q0c, mm0, mm1) ascending j:
      q0c: first q col (within block) to compute, mm0..mm1: mask-mul range
      (None if block fully valid over [q0c, QB)).
    """
    blocks = []
    for qb in range(NQB):
        entries = []
        for j in range(NKT):
            blk = m2[qb * QB:(qb + 1) * QB, j * 128:(j + 1) * 128]
            col_any = blk.any(axis=1)
            if not col_any.any():
                continue
            col_all = blk.all(axis=1)
            q0 = int(np.argmax(col_any))
            # q1: start of the trailing fully-valid run
            rev = col_all[::-1]
            run = int(np.argmin(rev)) if not rev.all() else QB
            q1 = QB - run
            if q1 <= q0:
                entries.append((j, q0, None, None))
            else:
                entries.append((j, q0, q0, q1))
        if entries:
            qmin = min(e[1] for e in entries)
            j, q0, m0, m1 = entries[0]
            if q0 > qmin:
                # first entry must cover every column later entries write
                entries[0] = (j, qmin, qmin, m1 if m1 is not None else q0)
        blocks.append(entries)
    return blocks


def _build(mask_bool):
    from contextlib import ExitStack
    import concourse.bass as bass
    import concourse.tile as tile
    from concourse import bacc, mybir

    f32 = mybir.dt.float32
    f32r = mybir.dt.float32r
    bf16 = mybir.dt.bfloat16
    EXP = mybir.ActivationFunctionType.Exp
    scale = 1.0 / np.sqrt(DK)

    m2 = mask_bool
    blocks = _analyze_mask(m2)

    nc = bacc.Bacc("TRN2", target_bir_lowering=False, debug=False)
    # xt is host-pre-tiled: tile (kd, c) = rows [(kd*NCH+c)*128, +128) so each
    # [128,512] DMA reads a fully contiguous 128KB DRAM region
    xt_d = nc.dram_tensor("xt", [NDT * NCH * 128, 512], bf16,
                          kind="ExternalInput")
    wqk_d = nc.dram_tensor("wqk", [D, 4 * 128], bf16, kind="ExternalInput")
    wv_d = nc.dram_tensor("wv", [D, 2 * 128], bf16, kind="ExternalInput")
    wo_d = nc.dram_tensor("wo", [2 * 128, D], bf16, kind="ExternalInput")
    mt_d = nc.dram_tensor("mt", [S, S], bf16, kind="ExternalInput")
    y_d = nc.dram_tensor("y", [T, D], bf16, kind="ExternalOutput")
    import os as _os
    dump = bool(_os.environ.get("KERNEL_DUMP"))
    if dump:
        qk_dump = nc.dram_tensor("qk_dump", [512, T], bf16, kind="ExternalOutput")
        v_dump = nc.dram_tensor("v_dump", [128, (T // 128) * 256], bf16,
                                kind="ExternalOutput")
        at_dump = nc.dram_tensor("at_dump", [256, T], bf16, kind="ExternalOutput")

    with tile.TileContext(nc) as tc:
        with ExitStack() as stack:
            stack.enter_context(
                nc.allow_low_precision(reason="float32r matmul inputs"))
            qkt_pool = stack.enter_context(tc.tile_pool(name="qkt", bufs=1))
            v_pool = stack.enter_context(tc.tile_pool(name="vsb", bufs=1))
            att_pool = stack.enter_context(tc.tile_pool(name="att", bufs=1))
            cst_pool = stack.enter_context(tc.tile_pool(name="cst", bufs=1))

            # persistent SBUF
            qt_sb = [qkt_pool.tile([128, T], bf16, tag=f"qt{h}", name=f"qt{h}")
                     for h in range(HPC)]
            kt_sb = [qkt_pool.tile([128, T], bf16, tag=f"kt{h}", name=f"kt{h}")
                     for h in range(HPC)]
            v_sb = v_pool.tile([128, (T // 128) * 256], bf16, tag="v")
            at_sb = [att_pool.tile([128, T], bf16, tag=f"at{h}", name=f"at{h}")
                     for h in range(HPC)]

            wo_pool = stack.enter_context(tc.tile_pool(name="wo", bufs=1))
            wo_sb = [wo_pool.tile([128, D], bf16, tag=f"wo{h}", name=f"wo{h}")
                     for h in range(HPC)]

            ones_f = cst_pool.tile([128, 128], f32, tag="ones_f")
            nc.vector.memset(ones_f[:], 1.0)
            ones_sq = cst_pool.tile([128, 128], bf16, tag="ones_sq")
            nc.scalar.copy(ones_sq[:], ones_f[:])

            # ---------------- phase 1: QKV projection ----------------
            with ExitStack() as p1:
                wqk_pool = p1.enter_context(tc.tile_pool(name="wqk", bufs=1))
                wv_pool = p1.enter_context(tc.tile_pool(name="wv", bufs=1))
                xt_pool = p1.enter_context(tc.tile_pool(name="xt", bufs=8))
                qk_ps_pool = p1.enter_context(
                    tc.tile_pool(name="ps_qk", bufs=4, space="PSUM"))
                v_ps_pool = p1.enter_context(
                    tc.tile_pool(name="ps_v", bufs=4, space="PSUM"))

                # weight DMAs are emitted inside the first chunk's kd loop so
                # the PE starts after ~one tile of DMA instead of the full
                # weight preload
                wqk_sb = [None] * NDT
                wv_sb = [None] * NDT

                for c in range(NCH):
                    qk_ps = [qk_ps_pool.tile([128, 512], f32, tag="qk", name="qkps")
                             for _ in range(4)]
                    v_ps = [v_ps_pool.tile([128, 256], f32, tag="v", name="vps")
                            for _ in range(4)]
                    for kd in range(NDT):
                        if c == 0:
                            wq = wqk_pool.tile([128, 512], bf16, tag=f"wqk{kd}")
                            nc.sync.dma_start(
                                wq[:], wqk_d.ap()[kd * 128:(kd + 1) * 128, :])
                            wqk_sb[kd] = wq
                        xt_t = xt_pool.tile([128, 512], bf16, tag="xt")
                        r0 = (kd * NCH + c) * 128
                        nc.sync.dma_start(xt_t[:], xt_d.ap()[r0:r0 + 128, :])
                        if c == 0:
                            wv_t = wv_pool.tile([128, 256], bf16, tag=f"wv{kd}")
                            nc.sync.dma_start(
                                wv_t[:], wv_d.ap()[kd * 128:(kd + 1) * 128, :])
                            wv_sb[kd] = wv_t
                        st, sp = kd == 0, kd == NDT - 1
                        for e in range(4):
                            nc.tensor.matmul(
                                qk_ps[e][:], wqk_sb[kd][:, e * 128:(e + 1) * 128],
                                xt_t[:], start=st, stop=sp)
                        for tl in range(4):
                            nc.tensor.matmul(
                                v_ps[tl][:],
                                xt_t[:, tl * 128:(tl + 1) * 128],
                                wv_sb[kd][:], start=st, stop=sp)
                    dsts = [qt_sb[0], qt_sb[1], kt_sb[0], kt_sb[1]]
                    for e in range(4):
                        nc.vector.tensor_copy(
                            dsts[e][:, c * 512:(c + 1) * 512], qk_ps[e][:])
                    for tl in range(4):
                        tok = c * 4 + tl
                        nc.scalar.copy(
                            v_sb[:, tok * 256:(tok + 1) * 256], v_ps[tl][:])

            # ---------------- phase 2 + 3: attention + projection ----------------
            with ExitStack() as p2:
                # e bufs=6: the G-chain stash defers one read by a round; the
                # pool must not recycle a stashed tile before that read emits
                e_pool = p2.enter_context(tc.tile_pool(name="e", bufs=6))
                acc_pool = p2.enter_context(tc.tile_pool(name="acc", bufs=4))
                rcp_pool = p2.enter_context(tc.tile_pool(name="rcp", bufs=2))
                msk_pool = p2.enter_context(tc.tile_pool(name="msk", bufs=1))
                ysb_pool = p2.enter_context(tc.tile_pool(name="ysb", bufs=4))
                # PSUM budget (8 banks of [128,512]f32): s 2 + o 2 + d 2 + y 2
                s_ps_pool = p2.enter_context(
                    tc.tile_pool(name="ps_s", bufs=2, space="PSUM"))
                o_ps_pool = p2.enter_context(
                    tc.tile_pool(name="ps_o", bufs=2, space="PSUM"))
                d_ps_pool = p2.enter_context(
                    tc.tile_pool(name="ps_d", bufs=2, space="PSUM"))
                y_ps_pool = p2.enter_context(
                    tc.tile_pool(name="ps_y", bufs=2, space="PSUM"))

                # W_o loads deferred to here: they are first needed a few µs
                # into phase 2, and emitting them up front would delay the
                # phase-1 weight/x DMAs that gate the first matmul.
                for h in range(HPC):
                    nc.sync.dma_start(wo_sb[h][:],
                                      wo_d.ap()[h * 128:(h + 1) * 128, :])

                # mask tile cache keyed by block content
                mask_tiles = {}

                def mask_tile(j, qb, m0, m1):
                    key = m2[qb * QB + m0:qb * QB + m1,
                             j * 128:(j + 1) * 128].tobytes()
                    t = mask_tiles.get(key)
                    if t is None:
                        t = msk_pool.tile([128, QB], bf16, name=f"mask{len(mask_tiles)}",
                                          tag=f"m{len(mask_tiles)}")
                        nc.sync.dma_start(
                            t[:, 0:m1 - m0],
                            mt_d.ap()[j * 128:(j + 1) * 128,
                                      qb * QB + m0:qb * QB + m1])
                        mask_tiles[key] = t
                    return t

                # Attention: the two head-streams of a batch are emitted
                # entry-interleaved (h0/h1 alternating per k-tile) so the PE
                # queue never blocks on one stream's exp; projection tiles
                # are emitted one q-block behind the attention that produces
                # their inputs, so their dependencies are ready when the
                # in-order PE queue reaches them. The softmax denominator is
                # accumulated on the PE itself (an all-ones stationary matmul
                # per entry into a dedicated PSUM bank, already broadcast
                # across partitions) — this keeps the DVE out of the
                # per-entry path entirely; the tail is just a fast approx
                # reciprocal + one multiply.
                class QbStream:
                    def __init__(self, b, h, qb):
                        self.b, self.h, self.qb = b, h, qb
                        self.tb = b * S
                        self.entries = blocks[qb]
                        self.ne = len(self.entries)
                        self.o_ps = o_ps_pool.tile([128, QB], f32, tag="o",
                                                   name="ops")
                        self.d_ps = d_ps_pool.tile([128, QB], f32, tag="d",
                                                   name="dps")
                        self.qcol = self.tb + qb * QB
                        self.pend = None
                        self.gi = 0
                        # most denominator entries accumulate on the PE
                        # (all-ones stationary matmul into d_ps, broadcast
                        # across partitions); early even full-width entries
                        # are offloaded to a GpSimd chain whose per-partition
                        # sum the tail folds into d_ps with one extra matmul.
                        # DVE stays off the per-entry path entirely.
                        assign = []
                        for gi, (j, q0c, m0, m1) in enumerate(self.entries):
                            if (DSPLIT and self.ne > 4 and q0c == 0
                                    and gi < self.ne - 6 and gi % 3 != 2):
                                assign.append('G')
                            else:
                                assign.append('P')
                        if assign.count('G') < 2:
                            # a chain needs >=2 entries (stash + 3-addr add)
                            assign = ['P' if a == 'G' else a for a in assign]
                        self.assign = assign
                        self.has_g = 'G' in assign
                        self.accG = (acc_pool.tile([128, QB], bf16, tag="accG",
                                                   name="accG")
                                     if self.has_g else None)
                        pidx = [i for i, a in enumerate(assign) if a == 'P']
                        self.p_first = pidx[0]
                        self.p_last = pidx[-1]
                        self.stashG = None
                        self.startedG = False

                    def s_and_exp(self, ent, gi):
                        j, q0c, m0, m1 = ent
                        s_ps = s_ps_pool.tile([128, QB], f32, tag="s",
                                              name="sps")
                        nc.tensor.matmul(
                            s_ps[:, q0c:QB],
                            kt_sb[self.h][:, self.tb + j * 128:
                                          self.tb + (j + 1) * 128],
                            qt_sb[self.h][:, self.qcol + q0c:
                                          self.qcol + QB],
                            start=True, stop=True)
                        e_sb = e_pool.tile([128, QB], bf16, tag="e",
                                           name="esb")
                        nc.scalar.activation(
                            e_sb[:, q0c:QB], s_ps[:, q0c:QB], EXP,
                            scale=scale)
                        if m0 is not None:
                            mtile = mask_tile(j, self.qb, m0, m1)
                            nc.vector.tensor_mul(
                                e_sb[:, m0:m1], e_sb[:, m0:m1],
                                mtile[:, 0:m1 - m0])
                        if self.assign[gi] == 'G':
                            # G entries are full-width by construction
                            if not self.startedG:
                                if self.stashG is None:
                                    self.stashG = e_sb[:, 0:QB]
                                else:
                                    nc.gpsimd.tensor_add(
                                        self.accG[:], self.stashG,
                                        e_sb[:, 0:QB])
                                    self.stashG = None
                                    self.startedG = True
                            else:
                                nc.gpsimd.tensor_add(self.accG[:],
                                                     self.accG[:],
                                                     e_sb[:, 0:QB])
                        return e_sb

                    def pv_and_d(self, ent, gi, e_sb):
                        j, q0c, m0, m1 = ent
                        nc.tensor.matmul(
                            self.o_ps[:, q0c:QB],
                            v_sb[:, (self.b * NKT + j) * 256 + self.h * 128:
                                 (self.b * NKT + j) * 256 + (self.h + 1) * 128],
                            e_sb[:, q0c:QB],
                            start=gi == 0, stop=gi == self.ne - 1)
                        if self.assign[gi] == 'P':
                            nc.tensor.matmul(
                                self.d_ps[:, q0c:QB], ones_sq[:],
                                e_sb[:, q0c:QB],
                                start=gi == self.p_first,
                                stop=(not self.has_g and gi == self.p_last))

                    def step(self):
                        # S+exp for entry gi, PV+denominator for entry gi-1
                        if self.gi < self.ne:
                            ent = self.entries[self.gi]
                            e_sb = self.s_and_exp(ent, self.gi)
                            if self.pend is not None:
                                self.pv_and_d(*self.pend)
                            self.pend = (ent, self.gi, e_sb)
                            self.gi += 1
                            return True
                        return False

                    def flush_pv(self):
                        if self.pend is not None:
                            self.pv_and_d(*self.pend)
                            self.pend = None

                    def tail(self):
                        # fold the GpSimd chain (per-partition entry sums)
                        # into the d_ps accumulation group with one all-ones
                        # matmul, then ~18-bit approx reciprocal (plenty for
                        # the 2e-2 tolerance) and the 1/d scaling multiply
                        if self.has_g:
                            nc.tensor.matmul(self.d_ps[:], ones_sq[:],
                                             self.accG[:],
                                             start=False, stop=True)
                        rcp = rcp_pool.tile([128, QB], f32, tag="rcp",
                                            name="rcp")
                        nc.vector.reciprocal_approx_fast(rcp[:], self.d_ps[:])
                        nc.vector.tensor_mul(
                            at_sb[self.h][:, self.qcol:self.qcol + QB],
                            self.o_ps[:], rcp[:])

                # y-copy engine rotation balances PSUM->SBUF cast traffic
                # across Scalar/DVE (GpSimd cannot read PSUM)
                y_engs = [nc.scalar, nc.vector, nc.vector, nc.scalar,
                          nc.vector, nc.scalar, nc.vector, nc.vector]
                y_cnt = [0]

                def emit_proj_tile(b, tt, force_scalar=False):
                    trow = (b * NKT + tt) * 128
                    for ch in range(4):
                        y_ps = y_ps_pool.tile([128, 512], f32, tag="y",
                                              name="yps")
                        for hh in range(HPC):
                            nc.tensor.matmul(
                                y_ps[:],
                                at_sb[hh][:, trow:trow + 128],
                                wo_sb[hh][:, ch * 512:(ch + 1) * 512],
                                start=(hh == 0), stop=(hh == HPC - 1))
                        y_sb = ysb_pool.tile([128, 512], bf16, tag="ysb",
                                             name="ysb")
                        eng = y_engs[y_cnt[0] % len(y_engs)]
                        y_cnt[0] += 1
                        if force_scalar:
                            # tail drains must not queue DVE work ahead of the
                            # reciprocal/scale ops the next block waits on
                            eng = nc.scalar
                        if eng is nc.scalar:
                            eng.copy(y_sb[:], y_ps[:])
                        else:
                            eng.tensor_copy(y_sb[:], y_ps[:])
                        nc.sync.dma_start(
                            y_d.ap()[trow:trow + 128,
                                     ch * 512:(ch + 1) * 512], y_sb[:])

                proj_queue = []  # (b, tt) pending projection tiles

                def drain_proj(n, reserve=0, force_scalar=False):
                    while n > 0 and len(proj_queue) > reserve:
                        emit_proj_tile(*proj_queue.pop(0), force_scalar)
                        n -= 1

                for b in range(B):
                    for qb in range(NQB):
                        # no reserve needed on the last block: nothing follows
                        # its tail, so drain eagerly to shorten the end
                        rsv = 0 if (b == B - 1 and qb == NQB - 1) else 2
                        streams = [QbStream(b, h, qb) for h in range(HPC)]
                        alive = True
                        while alive:
                            alive = False
                            for st in streams:
                                if st.step():
                                    alive = True
                            # keep proj tiles in reserve: they feed the PE
                            # at the block tail while the denominator chain
                            # drains
                            drain_proj(1, reserve=rsv)
                        for st in streams:
                            st.flush_pv()
                        drain_proj(2, force_scalar=True)
                        for st in streams:
                            st.tail()
                        proj_queue.extend((b, qb * 4 + t4) for t4 in range(4))
                drain_proj(len(proj_queue))

            if True:
                if dump:
                    dsts = [qt_sb[0], qt_sb[1], kt_sb[0], kt_sb[1]]
                    for e in range(4):
                        nc.sync.dma_start(
                            qk_dump.ap()[e * 128:(e + 1) * 128, :], dsts[e][:])
                    nc.sync.dma_start(v_dump.ap()[:, :], v_sb[:])
                    for h in range(HPC):
                        nc.sync.dma_start(
                            at_dump.ap()[h * 128:(h + 1) * 128, :], at_sb[h][:])
    nc.compile()
    return nc


last_results = None  # set when KERNEL_TRACE=1 (profiling from test harness)


def kernel(x, mask, W_qkv, W_o):
    import os
    from concourse.bass_utils import run_bass_kernel_spmd

    x = np.asarray(x, dtype=np.float32)
    mask_np = np.asarray(mask).astype(bool)
    W_qkv = np.asarray(W_qkv, dtype=np.float32)
    W_o = np.asarray(W_o, dtype=np.float32)
    m2 = np.broadcast_to(mask_np, (1, 1, S, S))[0, 0]

    key = m2.tobytes()
    nc = _cache.get(key)
    if nc is None:
        nc = _build(m2)
        _cache[key] = nc

    import ml_dtypes
    bf16 = ml_dtypes.bfloat16
    # [D, T] -> tiled [NDT, NCH, 128, 512] so each on-device tile is one
    # contiguous DRAM region
    xt = np.ascontiguousarray(
        x.reshape(T, D).T.astype(bf16)
        .reshape(NDT, 128, NCH, 512).transpose(0, 2, 1, 3)
        .reshape(NDT * NCH * 128, 512))
    mt = np.ascontiguousarray(m2.T.astype(bf16))                # [k, q]

    in_maps = []
    for c in range(NCORES):
        hA, hB = HPC * c, HPC * c + 1
        q_rows = list(range(hA * DK, (hA + 1) * DK)) + \
                 list(range(hB * DK, (hB + 1) * DK))
        k_rows = [D + r for r in q_rows]
        v_rows = [2 * D + r for r in q_rows]
        wqk = np.ascontiguousarray(W_qkv[q_rows + k_rows, :].T.astype(bf16))
        wv = np.ascontiguousarray(W_qkv[v_rows, :].T.astype(bf16))
        wo = np.ascontiguousarray(W_o[:, q_rows].T.astype(bf16))
        in_maps.append({"xt": xt, "wqk": wqk, "wv": wv, "wo": wo, "mt": mt})

    trace = bool(os.environ.get("KERNEL_TRACE"))
    res = run_bass_kernel_spmd(nc, in_maps, core_ids=list(range(NCORES)),
                               trace=trace)
    if trace:
        global last_results
        last_results = res
    y = res.results[0]["y"].astype(np.float32)
    for c in range(1, NCORES):
        y += res.results[c]["y"].astype(np.float32)
    return y.reshape(B, S, D)



# revision 62
# speedup vs baseline: 1.0147x; 1.0147x over previous
"""Multi-head causal attention on 8 Trainium2 NeuronCores.

nn_MultiHeadAttention_37933151158277: x[2,2048,2048] f32, causal mask,
W_qkv[6144,2048], W_o[2048,2048]. Tensor-parallel over heads (2 per
core), per the sharding hint: qkv_proj output and W_o input are split
along the head dimension; x is replicated. Each core:

  phase 1 - QKV projection, all inputs in bf16 (same 1-cycle/row PE
      rate as f32r, half the DMA bytes; quantization adds ~2e-3 to the
      end-to-end error, far under the 2e-2 gate). Host supplies x^T
      pre-tiled ([kd, c] tiles contiguous in DRAM) and per-core weight
      slices pre-transposed, so Q^T/K^T land as [d_k=128, tok] and V
      as [tok, d_k] with zero on-device transposes. Weight DMAs are
      emitted inside the first chunk's kd loop, interleaved with the
      x^T tile loads, so the PE starts after ~one tile of DMA instead
      of the full preload; W_o loads are deferred to phase 2.
  phase 2 - attention per (batch, head). Scores are computed
      transposed: S^T[k, q] = K^T_tile.T @ Q^T (contraction over d), so
      the P @ V matmul can consume exp(S^T) directly with V tiles as
      the stationary operand. No max-subtraction (scores are O(1) by
      construction, exp cannot overflow). The softmax denominator is
      accumulated mostly ON THE PE: an all-ones [128,128] stationary
      matmul per entry into a dedicated PSUM bank yields the
      partition-reduced denominator already broadcast across all 128
      partitions; early full-width entries are offloaded to a GpSimd
      add-chain whose per-partition sum the q-block tail folds into
      the same PSUM accumulation group with one extra ones-matmul.
      The tail is then just reciprocal_approx_fast (~18-bit, on all
      128 DVE lanes) + one o_ps x 1/d multiply - nothing serial
      enough to stall the in-order PE queue, which matters doubly
      because the PE drops out of its 2.4GHz p-state after any gap
      (mid p-state is 1.2GHz for the next 3us). Two head-streams are
      emitted entry-interleaved; 2 proj tiles are held in reserve to
      feed the PE across each block tail.
  phase 3 - partial out-projection y_c = attn_out @ W_o[:, cols]^T,
      drained one q-block behind attention; PSUM->SBUF copies rotate
      between Scalar and DVE so neither queue delays exp/reciprocal;
      y is written in bf16 (halves output DMA; negligible error since
      the host reduces the 8 partial sums in f32).

PSUM budget (8 x 2KB banks): s_ps 2 + o_ps 2 + d_ps 2 + y_ps 2.

Host: y = sum_c y_c (the unshard of the head-parallel partial sums).

The mask is analyzed block-wise at trace time: fully-masked blocks are
skipped, fully-valid blocks skip the mask multiply, mixed blocks get a
(content-deduped) DMA'd mask-tile multiply - for the causal mask this
yields the optimal lower-triangular schedule with a single shared
128x128 triangle tile.

fp8 was measured (numpy e4m3 emulation) and rejected: every variant
(scores-only, PV-only, projections) lands at 0.9e-2..6.4e-2 absmax -
at or over the 2e-2 gate - so matmuls stay bf16/f32r at 1 cycle/row.

Phase 2 also runs entirely in bf16 (qt/kt/v/e/mask/at/W_o): same
1-cycle/row PE rate, but half the SBUF/LDWEIGHTS traffic (matmul
slices deflate measurably) and 2x DVE mask-multiply rate. exp's
bf16 quantization cancels between the PV numerator and the
denominator chains, which read the same quantized values.

Measured: 358702 ns HW exec (8 cores SPMD), rel err 4.3e-3
(baseline inherited from the previous session: 462552 ns / 2.5e-4;
note ~1-2%% run-to-run device clock variance was observed).
"""
import sys
if '/opt/trn_rl_repo' not in sys.path:
    sys.path.insert(0, '/opt/trn_rl_repo')

import numpy as np

B, S, D = 2, 2048, 2048
H, DK = 16, 128
NCORES = 8
HPC = H // NCORES            # heads per core
T = B * S                    # tokens
QB = 512                     # q-block width (free dim of S^T / PV matmuls)
NKT = S // 128               # k tiles per batch (16)
NQB = S // QB                # q blocks per batch (4)
NCH = T // QB                # token chunks (8)
NDT = D // 128               # d_model tiles (16)
SGRP = 1                     # k-tiles per s-psum group

_cache = {}
DSPLIT = True


def _analyze_mask(m2):
    """m2: [S, S] bool, m2[q, k]. Returns blocks[qb] = list of entries
    (j, q0c, mm0, mm1) ascending j:
      q0c: first q col (within block) to compute, mm0..mm1: mask-mul range
      (None if block fully valid over [q0c, QB)).
    """
    blocks = []
    for qb in range(NQB):
        entries = []
        for j in range(NKT):
            blk = m2[qb * QB:(qb + 1) * QB, j * 128:(j + 1) * 128]
            col_any = blk.any(axis=1)
            if not col_any.any():
                continue
            col_all = blk.all(axis=1)
            q0 = int(np.argmax(col_any))
            # q1: start of the trailing fully-valid run
            rev = col_all[::-1]
            run = int(np.argmin(rev)) if not rev.all() else QB
            q1 = QB - run
            if q1 <= q0:
                entries.append((j, q0, None, None))
            else:
                entries.append((j, q0, q0, q1))
        if entries:
            qmin = min(e[1] for e in entries)
            j, q0, m0, m1 = entries[0]
            if q0 > qmin:
                # first entry must cover every column later entries write
                entries[0] = (j, qmin, qmin, m1 if m1 is not None else q0)
        blocks.append(entries)
    return blocks


def _build(mask_bool):
    from contextlib import ExitStack
    import concourse.bass as bass
    import concourse.tile as tile
    from concourse import bacc, mybir

    f32 = mybir.dt.float32
    f32r = mybir.dt.float32r
    bf16 = mybir.dt.bfloat16
    EXP = mybir.ActivationFunctionType.Exp
    scale = 1.0 / np.sqrt(DK)

    m2 = mask_bool
    blocks = _analyze_mask(m2)

    nc = bacc.Bacc("TRN2", target_bir_lowering=False, debug=False)
    # xt is host-pre-tiled: tile (kd, c) = rows [(kd*NCH+c)*128, +128) so each
    # [128,512] DMA reads a fully contiguous 128KB DRAM region
    xt_d = nc.dram_tensor("xt", [NDT * NCH * 128, 512], bf16,
                          kind="ExternalInput")
    wqk_d = nc.dram_tensor("wqk", [D, 4 * 128], bf16, kind="ExternalInput")
    wv_d = nc.dram_tensor("wv", [D, 2 * 128], bf16, kind="ExternalInput")
    wo_d = nc.dram_tensor("wo", [2 * 128, D], bf16, kind="ExternalInput")
    mt_d = nc.dram_tensor("mt", [S, S], bf16, kind="ExternalInput")
    y_d = nc.dram_tensor("y", [T, D], bf16, kind="ExternalOutput")
    import os as _os
    dump = bool(_os.environ.get("KERNEL_DUMP"))
    if dump:
        qk_dump = nc.dram_tensor("qk_dump", [512, T], bf16, kind="ExternalOutput")
        v_dump = nc.dram_tensor("v_dump", [128, (T // 128) * 256], bf16,
                                kind="ExternalOutput")
        at_dump = nc.dram_tensor("at_dump", [256, T], bf16, kind="ExternalOutput")

    with tile.TileContext(nc) as tc:
        with ExitStack() as stack:
            stack.enter_context(
                nc.allow_low_precision(reason="float32r matmul inputs"))
            qkt_pool = stack.enter_context(tc.tile_pool(name="qkt", bufs=1))
            v_pool = stack.enter_context(tc.tile_pool(name="vsb", bufs=1))
            att_pool = stack.enter_context(tc.tile_pool(name="att", bufs=1))
            cst_pool = stack.enter_context(tc.tile_pool(name="cst", bufs=1))

            # persistent SBUF
            qt_sb = [qkt_pool.tile([128, T], bf16, tag=f"qt{h}", name=f"qt{h}")
                     for h in range(HPC)]
            kt_sb = [qkt_pool.tile([128, T], bf16, tag=f"kt{h}", name=f"kt{h}")
                     for h in range(HPC)]
            v_sb = v_pool.tile([128, (T // 128) * 256], bf16, tag="v")
            at_sb = [att_pool.tile([128, T], bf16, tag=f"at{h}", name=f"at{h}")
                     for h in range(HPC)]

            wo_pool = stack.enter_context(tc.tile_pool(name="wo", bufs=1))
            wo_sb = [wo_pool.tile([128, D], bf16, tag=f"wo{h}", name=f"wo{h}")
                     for h in range(HPC)]

            ones_f = cst_pool.tile([128, 128], f32, tag="ones_f")
            nc.vector.memset(ones_f[:], 1.0)
            ones_sq = cst_pool.tile([128, 128], bf16, tag="ones_sq")
            nc.scalar.copy(ones_sq[:], ones_f[:])

            # ---------------- phase 1: QKV projection ----------------
            with ExitStack() as p1:
                wqk_pool = p1.enter_context(tc.tile_pool(name="wqk", bufs=1))
                wv_pool = p1.enter_context(tc.tile_pool(name="wv", bufs=1))
                xt_pool = p1.enter_context(tc.tile_pool(name="xt", bufs=8))
                qk_ps_pool = p1.enter_context(
                    tc.tile_pool(name="ps_qk", bufs=4, space="PSUM"))
                v_ps_pool = p1.enter_context(
                    tc.tile_pool(name="ps_v", bufs=4, space="PSUM"))

                # weight DMAs are emitted inside the first chunk's kd loop so
                # the PE starts after ~one tile of DMA instead of the full
                # weight preload
                wqk_sb = [None] * NDT
                wv_sb = [None] * NDT

                for c in range(NCH):
                    qk_ps = [qk_ps_pool.tile([128, 512], f32, tag="qk", name="qkps")
                             for _ in range(4)]
                    v_ps = [v_ps_pool.tile([128, 256], f32, tag="v", name="vps")
                            for _ in range(4)]
                    for kd in range(NDT):
                        if c == 0:
                            wq = wqk_pool.tile([128, 512], bf16, tag=f"wqk{kd}")
                            nc.sync.dma_start(
                                wq[:], wqk_d.ap()[kd * 128:(kd + 1) * 128, :])
                            wqk_sb[kd] = wq
                        xt_t = xt_pool.tile([128, 512], bf16, tag="xt")
                        r0 = (kd * NCH + c) * 128
                        nc.sync.dma_start(xt_t[:], xt_d.ap()[r0:r0 + 128, :])
                        if c == 0:
                            wv_t = wv_pool.tile([128, 256], bf16, tag=f"wv{kd}")
                            nc.sync.dma_start(
                                wv_t[:], wv_d.ap()[kd * 128:(kd + 1) * 128, :])
                            wv_sb[kd] = wv_t
                        st, sp = kd == 0, kd == NDT - 1
                        for e in range(4):
                            nc.tensor.matmul(
                                qk_ps[e][:], wqk_sb[kd][:, e * 128:(e + 1) * 128],
                                xt_t[:], start=st, stop=sp)
                        for tl in range(4):
                            nc.tensor.matmul(
                                v_ps[tl][:],
                                xt_t[:, tl * 128:(tl + 1) * 128],
                                wv_sb[kd][:], start=st, stop=sp)
                    dsts = [qt_sb[0], qt_sb[1], kt_sb[0], kt_sb[1]]
                    for e in range(4):
                        nc.vector.tensor_copy(
                            dsts[e][:, c * 512:(c + 1) * 512], qk_ps[e][:])
                    for tl in range(4):
                        tok = c * 4 + tl
                        nc.scalar.copy(
                            v_sb[:, tok * 256:(tok + 1) * 256], v_ps[tl][:])

            # ---------------- phase 2 + 3: attention + projection ----------------
            with ExitStack() as p2:
                # e bufs=6: the G-chain stash defers one read by a round; the
                # pool must not recycle a stashed tile before that read emits
                e_pool = p2.enter_context(tc.tile_pool(name="e", bufs=8))
                acc_pool = p2.enter_context(tc.tile_pool(name="acc", bufs=4))
                rcp_pool = p2.enter_context(tc.tile_pool(name="rcp", bufs=2))
                msk_pool = p2.enter_context(tc.tile_pool(name="msk", bufs=1))
                ysb_pool = p2.enter_context(tc.tile_pool(name="ysb", bufs=4))
                # PSUM budget (8 banks of [128,512]f32): s 2 + o 2 + d 2 + y 2
                s_ps_pool = p2.enter_context(
                    tc.tile_pool(name="ps_s", bufs=2, space="PSUM"))
                o_ps_pool = p2.enter_context(
                    tc.tile_pool(name="ps_o", bufs=2, space="PSUM"))
                d_ps_pool = p2.enter_context(
                    tc.tile_pool(name="ps_d", bufs=2, space="PSUM"))
                y_ps_pool = p2.enter_context(
                    tc.tile_pool(name="ps_y", bufs=2, space="PSUM"))

                # W_o loads deferred to here: they are first needed a few µs
                # into phase 2, and emitting them up front would delay the
                # phase-1 weight/x DMAs that gate the first matmul.
                for h in range(HPC):
                    nc.sync.dma_start(wo_sb[h][:],
                                      wo_d.ap()[h * 128:(h + 1) * 128, :])

                # mask tile cache keyed by block content
                mask_tiles = {}

                def mask_tile(j, qb, m0, m1):
                    key = m2[qb * QB + m0:qb * QB + m1,
                             j * 128:(j + 1) * 128].tobytes()
                    t = mask_tiles.get(key)
                    if t is None:
                        t = msk_pool.tile([128, QB], bf16, name=f"mask{len(mask_tiles)}",
                                          tag=f"m{len(mask_tiles)}")
                        nc.sync.dma_start(
                            t[:, 0:m1 - m0],
                            mt_d.ap()[j * 128:(j + 1) * 128,
                                      qb * QB + m0:qb * QB + m1])
                        mask_tiles[key] = t
                    return t

                # Attention: the two head-streams of a batch are emitted
                # entry-interleaved (h0/h1 alternating per k-tile) so the PE
                # queue never blocks on one stream's exp; projection tiles
                # are emitted one q-block behind the attention that produces
                # their inputs, so their dependencies are ready when the
                # in-order PE queue reaches them. The softmax denominator is
                # accumulated on the PE itself (an all-ones stationary matmul
                # per entry into a dedicated PSUM bank, already broadcast
                # across partitions) — this keeps the DVE out of the
                # per-entry path entirely; the tail is just a fast approx
                # reciprocal + one multiply.
                class QbStream:
                    def __init__(self, b, h, qb):
                        self.b, self.h, self.qb = b, h, qb
                        self.tb = b * S
                        self.entries = blocks[qb]
                        self.ne = len(self.entries)
                        self.o_ps = o_ps_pool.tile([128, QB], f32, tag="o",
                                                   name="ops")
                        self.d_ps = d_ps_pool.tile([128, QB], f32, tag="d",
                                                   name="dps")
                        self.qcol = self.tb + qb * QB
                        self.pend = None
                        self.gi = 0
                        # most denominator entries accumulate on the PE
                        # (all-ones stationary matmul into d_ps, broadcast
                        # across partitions); early even full-width entries
                        # are offloaded to a GpSimd chain whose per-partition
                        # sum the tail folds into d_ps with one extra matmul.
                        # DVE stays off the per-entry path entirely.
                        assign = []
                        for gi, (j, q0c, m0, m1) in enumerate(self.entries):
                            if (DSPLIT and self.ne > 4 and q0c == 0
                                    and gi < self.ne - 6 and gi % 3 != 2):
                                assign.append('G')
                            else:
                                assign.append('P')
                        if assign.count('G') < 2:
                            # a chain needs >=2 entries (stash + 3-addr add)
                            assign = ['P' if a == 'G' else a for a in assign]
                        self.assign = assign
                        self.has_g = 'G' in assign
                        self.accG = (acc_pool.tile([128, QB], bf16, tag="accG",
                                                   name="accG")
                                     if self.has_g else None)
                        pidx = [i for i, a in enumerate(assign) if a == 'P']
                        self.p_first = pidx[0]
                        self.p_last = pidx[-1]
                        self.stashG = None
                        self.startedG = False

                    def s_and_exp(self, ent, gi):
                        j, q0c, m0, m1 = ent
                        s_ps = s_ps_pool.tile([128, QB], f32, tag="s",
                                              name="sps")
                        nc.tensor.matmul(
                            s_ps[:, q0c:QB],
                            kt_sb[self.h][:, self.tb + j * 128:
                                          self.tb + (j + 1) * 128],
                            qt_sb[self.h][:, self.qcol + q0c:
                                          self.qcol + QB],
                            start=True, stop=True)
                        e_sb = e_pool.tile([128, QB], bf16, tag="e",
                                           name="esb")
                        nc.scalar.activation(
                            e_sb[:, q0c:QB], s_ps[:, q0c:QB], EXP,
                            scale=scale)
                        if m0 is not None:
                            mtile = mask_tile(j, self.qb, m0, m1)
                            nc.vector.tensor_mul(
                                e_sb[:, m0:m1], e_sb[:, m0:m1],
                                mtile[:, 0:m1 - m0])
                        if self.assign[gi] == 'G':
                            # G entries are full-width by construction
                            if not self.startedG:
                                if self.stashG is None:
                                    self.stashG = e_sb[:, 0:QB]
                                else:
                                    nc.gpsimd.tensor_add(
                                        self.accG[:], self.stashG,
                                        e_sb[:, 0:QB])
                                    self.stashG = None
                                    self.startedG = True
                            else:
                                nc.gpsimd.tensor_add(self.accG[:],
                                                     self.accG[:],
                                                     e_sb[:, 0:QB])
                        return e_sb

                    def pv_and_d(self, ent, gi, e_sb):
                        j, q0c, m0, m1 = ent
                        nc.tensor.matmul(
                            self.o_ps[:, q0c:QB],
                            v_sb[:, (self.b * NKT + j) * 256 + self.h * 128:
                                 (self.b * NKT + j) * 256 + (self.h + 1) * 128],
                            e_sb[:, q0c:QB],
                            start=gi == 0, stop=gi == self.ne - 1)
                        if self.assign[gi] == 'P':
                            nc.tensor.matmul(
                                self.d_ps[:, q0c:QB], ones_sq[:],
                                e_sb[:, q0c:QB],
                                start=gi == self.p_first,
                                stop=(not self.has_g and gi == self.p_last))

                    def step(self):
                        # S+exp for entry gi, PV+denominator for entry gi-1
                        if self.gi < self.ne:
                            ent = self.entries[self.gi]
                            e_sb = self.s_and_exp(ent, self.gi)
                            if self.pend is not None:
                                self.pv_and_d(*self.pend)
                            self.pend = (ent, self.gi, e_sb)
                            self.gi += 1
                            return True
                        return False

                    def flush_pv(self):
                        if self.pend is not None:
                            self.pv_and_d(*self.pend)
                            self.pend = None

                    def tail(self):
                        # fold the GpSimd chain (per-partition entry sums)
                        # into the d_ps accumulation group with one all-ones
                        # matmul, then ~18-bit approx reciprocal (plenty for
                        # the 2e-2 tolerance) and the 1/d scaling multiply
                        if self.has_g:
                            nc.tensor.matmul(self.d_ps[:], ones_sq[:],
                                             self.accG[:],
                                             start=False, stop=True)
                        rcp = rcp_pool.tile([128, QB], f32, tag="rcp",
                                            name="rcp")
                        nc.vector.reciprocal_approx_fast(rcp[:], self.d_ps[:])
                        nc.vector.tensor_mul(
                            at_sb[self.h][:, self.qcol:self.qcol + QB],
                            self.o_ps[:], rcp[:])

                # y-copy engine rotation balances PSUM->SBUF cast traffic
                # across Scalar/DVE (GpSimd cannot read PSUM)
                y_engs = [nc.scalar, nc.vector, nc.vector, nc.scalar,
                          nc.vector, nc.scalar, nc.vector, nc.vector]
                y_cnt = [0]

                def emit_proj_tile(b, tt):
                    trow = (b * NKT + tt) * 128
                    for ch in range(4):
                        y_ps = y_ps_pool.tile([128, 512], f32, tag="y",
                                              name="yps")
                        for hh in range(HPC):
                            nc.tensor.matmul(
                                y_ps[:],
                                at_sb[hh][:, trow:trow + 128],
                                wo_sb[hh][:, ch * 512:(ch + 1) * 512],
                                start=(hh == 0), stop=(hh == HPC - 1))
                        y_sb = ysb_pool.tile([128, 512], bf16, tag="ysb",
                                             name="ysb")
                        eng = y_engs[y_cnt[0] % len(y_engs)]
                        y_cnt[0] += 1
                        if eng is nc.scalar:
                            eng.copy(y_sb[:], y_ps[:])
                        else:
                            eng.tensor_copy(y_sb[:], y_ps[:])
                        nc.sync.dma_start(
                            y_d.ap()[trow:trow + 128,
                                     ch * 512:(ch + 1) * 512], y_sb[:])

                proj_queue = []  # (b, tt) pending projection tiles

                def drain_proj(n, reserve=0):
                    while n > 0 and len(proj_queue) > reserve:
                        emit_proj_tile(*proj_queue.pop(0))
                        n -= 1

                for b in range(B):
                    for qb in range(NQB):
                        # no reserve needed on the last block: nothing follows
                        # its tail, so drain eagerly to shorten the end
                        rsv = 0 if (b == B - 1 and qb == NQB - 1) else 2
                        streams = [QbStream(b, h, qb) for h in range(HPC)]
                        alive = True
                        nstep = 0
                        while alive:
                            alive = False
                            for st in streams:
                                if st.step():
                                    alive = True
                            nstep += 1
                            # keep proj tiles in reserve: they feed the PE
                            # at the block tail while the denominator chain
                            # drains. Skip round 1: the previous block's
                            # at_sb (rcp+mul on DVE) is not ready yet and a
                            # round-1 proj matmul would stall the PE on it.
                            if nstep >= 2:
                                drain_proj(1, reserve=rsv)
                        for st in streams:
                            st.flush_pv()
                        drain_proj(2)
                        for st in streams:
                            st.tail()
                        proj_queue.extend((b, qb * 4 + t4) for t4 in range(4))
                drain_proj(len(proj_queue))

            if True:
                if dump:
                    dsts = [qt_sb[0], qt_sb[1], kt_sb[0], kt_sb[1]]
                    for e in range(4):
                        nc.sync.dma_start(
                            qk_dump.ap()[e * 128:(e + 1) * 128, :], dsts[e][:])
                    nc.sync.dma_start(v_dump.ap()[:, :], v_sb[:])
                    for h in range(HPC):
                        nc.sync.dma_start(
                            at_dump.ap()[h * 128:(h + 1) * 128, :], at_sb[h][:])
    nc.compile()
    return nc


last_results = None  # set when KERNEL_TRACE=1 (profiling from test harness)


def kernel(x, mask, W_qkv, W_o):
    import os
    from concourse.bass_utils import run_bass_kernel_spmd

    x = np.asarray(x, dtype=np.float32)
    mask_np = np.asarray(mask).astype(bool)
    W_qkv = np.asarray(W_qkv, dtype=np.float32)
    W_o = np.asarray(W_o, dtype=np.float32)
    m2 = np.broadcast_to(mask_np, (1, 1, S, S))[0, 0]

    key = m2.tobytes()
    nc = _cache.get(key)
    if nc is None:
        nc = _build(m2)
        _cache[key] = nc

    import ml_dtypes
    bf16 = ml_dtypes.bfloat16
    # [D, T] -> tiled [NDT, NCH, 128, 512] so each on-device tile is one
    # contiguous DRAM region
    xt = np.ascontiguousarray(
        x.reshape(T, D).T.astype(bf16)
        .reshape(NDT, 128, NCH, 512).transpose(0, 2, 1, 3)
        .reshape(NDT * NCH * 128, 512))
    mt = np.ascontiguousarray(m2.T.astype(bf16))                # [k, q]

    in_maps = []
    for c in range(NCORES):
        hA, hB = HPC * c, HPC * c + 1
        q_rows = list(range(hA * DK, (hA + 1) * DK)) + \
                 list(range(hB * DK, (hB + 1) * DK))
        k_rows = [D + r for r in q_rows]
        v_rows = [2 * D + r for r in q_rows]
        wqk = np.ascontiguousarray(W_qkv[q_rows + k_rows, :].T.astype(bf16))
        wv = np.ascontiguousarray(W_qkv[v_rows, :].T.astype(bf16))
        wo = np.ascontiguousarray(W_o[:, q_rows].T.astype(bf16))
        in_maps.append({"xt": xt, "wqk": wqk, "wv": wv, "wo": wo, "mt": mt})

    trace = bool(os.environ.get("KERNEL_TRACE"))
    res = run_bass_kernel_spmd(nc, in_maps, core_ids=list(range(NCORES)),
                               trace=trace)
    if trace:
        global last_results
        last_results = res
    y = res.results[0]["y"].astype(np.float32)
    for c in range(1, NCORES):
        y += res.results[c]["y"].astype(np.float32)
    return y.reshape(B, S, D)

